# revision 15
# baseline (speedup 1.0000x reference)
"""Trainium2 Bass kernel for nn_Net_34359738709 (spiking RNN).

Model (per timestep t, reference semantics):
    cur1  = x_t @ W1.T + b1                      # [B, NH] big matmul, t-independent
    mem1  = beta1c*mem1 + cur1 + spk1 @ V.T + Vb - spk1*THRESH
    spk1  = (mem1 - THRESH > 0)
    cur2  = spk1 @ W2.T + b2
    mem2  = BETA2*mem2 + cur2 - spk2_prev*THRESH
    spk2  = (mem2 - THRESH > 0)
outputs: (spk2_rec, mem2_rec), each [T, B, NOUT]

== Stream phase ==
Data-parallel over batch (B=64 -> 8 cores x 8). cur1.T[NH, 400] is ONE
400-column tile accumulated over 256 K-chunks of 128 in PSUM. Precision
scheme (numpy-validated exact spike pattern, margin 1.2e-5):
    x   = fp16(x)  +  2^-12 * e3m4((x - fp16(x)) * 2^12)     3 B/elem
    W1  = fp16(W1) + 2^-21 * e3m4((W1 - fp16(W1)) * 2^21)    4 B/elem
    T1 = w16.T @ x16 (fp16);  T2 = wlo8.T @ x16 (e3m4 lhsT x fp16 rhs,
    mixed dtype);  T3 = w8.T @ xlo8 (e3m4, w8 = e3m4(W1*2^9)).
T2/T3 share scale 2^21 -> one shared psum; cur1 = psA + 2^-21*psB + b1.
W1 planes are NH-padded to 128 columns so LDWEIGHTS hits the
compiler's Fast-Weight-Load path, and streamed through 16-chunk pool
slices on the scalar ring; x16+xlo8 stream on the sync ring.

== Scan phase (unified, 51 steps) ==
Both layers run in ONE recurrence on a 128-row state vector
s = [spk1(0..99); spk2(100..110); 1(111); step>0(112)]:
    A[:, 0:100]   = beta1c * [(V-I).T; Vb at row 111]          (layer 1)
    A[:, 100:111] = BETA2 * [W2.T; -I at rows 100..110; b2 at row 112]
    (layer 2 rides at lag 1: its "input current" c2 - reset arrives
     through the SAME matmul; row 112 gates b2 off at t=0)
    u_{t+1} = beta (.) u_t - d1_t,   d1_t = A.T @ s_t + cur_{t+1}
    s_{t+1} = (u_{t+1} < -beta - cur_{t+1})     [one is_lt vs a table]
    mem2_t  = -u2_{t+2} / BETA2                  [recovered post-loop]
Per step: one fp16 matmul pair (A16 + Alo, Alo held as raw fp16
SUBNORMALS - HW-verified exact) accumulating onto a cur-preloaded PSUM
(preload = one fp32 identity matmul), then TWO DVE ops. No fp32 scan
matmuls, no separate layer-2 chain, no W2 bursts, no mem1 tensor.

Negative results (measured, don't re-try): 3-column-tile pipeline
(228us: 3x LDWEIGHTS re-tax, scan convoys block later tiles' MMs on
the in-order PE queue; any multi-tile split loses); fp32 scan matmul
(2-pass lowering, 288-338ns LDW + 353ns MM per step); layer-2 lag-8
interleave on DVE (6 DVE ops/step -> 1.3us/step); layer-1/layer-2
fixpoint-iteration scans (46/12 iterations on the seed data - causal
spike chains are deep); bf16x2 4B/elem split (233us); unpadded NH=100
stationary tiles; scan steps interleaved into MM groups; GpSimd.
"""

import sys

if "/opt/trn_rl_repo" not in sys.path:
    sys.path.insert(0, "/opt/trn_rl_repo")

import numpy as np

# Problem shapes (hardcoded per contract)
T, B, NIN, NH, NOUT = 50, 64, 32768, 100, 11
NCORES = 8
BL = B // NCORES          # 8 batch rows per core
TBL = T * BL              # 400 columns (t-major: col = t*BL + b)
KP = 128                  # contraction partition size
NHP = 128                 # NH padded to 128 (stationary tile width for FWL)
KCH = NIN // KP           # 256 K-chunks
X_GROUPS = [2, 2, 4, 8] + [16] * 15   # K-chunks per x dma_start (sums to 256)
W_SLICE = 16              # K-chunks per W1 dma slice
THRESH = 1.0
BETA2 = 0.9753
NU = NH + NOUT            # 111 unified state rows
NSTEP = T + 1             # 51 scan steps (last one finishes layer 2)
SBLK = NSTEP + 1          # 52 column blocks in scan tables

SC_XLO = 2.0 ** 12        # xlo8 = e3m4((x - x16) * SC_XLO)
SC_WLO = 2.0 ** 21        # wlo8 = e3m4((W1 - w16) * SC_WLO)
SC_W8 = 2.0 ** 9          # w8   = e3m4(W1 * SC_W8); SC_W8*SC_XLO == SC_WLO
COMB = 1.0 / SC_WLO       # psumB combine scale

_PROG = {}


def _build_body(tc, nc, mybir, aps):
    f32 = mybir.dt.float32
    f16 = mybir.dt.float16
    e3 = mybir.dt.float8e3
    Alu = mybir.AluOpType
    (xt16, xt8, w16f, wlo8f, w8f, a16d, alod, id32d, bcold, betnd, b1d,
     s1initd, spk_o, mem_o) = aps

    from contextlib import ExitStack

    stack = ExitStack()
    const_pool = stack.enter_context(tc.tile_pool(name="const", bufs=1))
    state_pool = stack.enter_context(tc.tile_pool(name="state", bufs=1))
    w16pool = stack.enter_context(tc.tile_pool(name="w16p", bufs=4))
    wlo8pool = stack.enter_context(tc.tile_pool(name="wlo8p", bufs=4))
    w8pool = stack.enter_context(tc.tile_pool(name="w8p", bufs=4))
    xpool16 = stack.enter_context(tc.tile_pool(name="xg16", bufs=6))
    xpool8 = stack.enter_context(tc.tile_pool(name="xg8", bufs=6))
    ps_a = stack.enter_context(tc.tile_pool(name="psa", bufs=1, space="PSUM"))
    ps_b = stack.enter_context(tc.tile_pool(name="psb", bufs=1, space="PSUM"))
    ps_s = stack.enter_context(tc.tile_pool(name="pss", bufs=1, space="PSUM"))

    MAXG = max(X_GROUPS)

    # ---- scan constants / tables ----
    a16sb = const_pool.tile([KP, KP], f16)    # unified A matrix, fp16 hi
    alosb = const_pool.tile([KP, KP], f16)    # fp16 lo (subnormals)
    id32sb = const_pool.tile([KP, KP], f32)   # identity (psum preload)
    bcolsb = const_pool.tile([NU, 1], f32)    # [beta1c; BETA2] positive
    betnsb = const_pool.tile([NH, 1], f32)    # -beta1c
    b1sb = const_pool.tile([NH, 1], f32)
    spk1buf = state_pool.tile([KP, SBLK * BL], f16)  # unified spike state
    curt = state_pool.tile([KP, SBLK * BL], f32)     # cur1 table (+0 pad)
    thtab = state_pool.tile([NU, SBLK * BL], f32)    # threshold table
    utab = state_pool.tile([NU, SBLK * BL], f32)     # u state per step
    memout = state_pool.tile([4 + NOUT, TBL], f32)  # rows 4..14 are mem2

    def load_consts():
        nc.scalar.dma_start(a16sb[:], a16d)
        nc.scalar.dma_start(alosb[:], alod)
        nc.scalar.dma_start(id32sb[:], id32d)
        nc.scalar.dma_start(bcolsb[:], bcold)
        nc.scalar.dma_start(betnsb[:], betnd)
        nc.scalar.dma_start(b1sb[:], b1d)
        nc.scalar.dma_start(spk1buf[:], s1initd)

    # zero the pad regions of curt once (rows 100..127 everywhere, and
    # the two trailing column blocks of the cur rows)
    # (DVE partition starts must be 32-aligned; rows 96..99 of each
    # pad-memset are overwritten by the later 0:NH writes)
    nc.vector.memset(curt[96:KP, :], 0.0)
    nc.vector.memset(curt[0:NH, TBL:], 0.0)
    nc.vector.memset(thtab[96:NU, :], -BETA2)
    nc.vector.memset(utab[96:NU, 0:BL], 0.0)

    # ---- streaming matmul: one pass over 256 K-chunks, 400 columns ----
    psa = ps_a.tile([NHP, TBL], f32)
    psb = ps_b.tile([NHP, TBL], f32)
    w16t = wlo8t = w8t = None
    c0 = 0
    for g, gch in enumerate(X_GROUPS):
        if c0 % W_SLICE == 0:
            ws = c0 // W_SLICE
            w16t = w16pool.tile([KP, W_SLICE * NHP], f16)
            wlo8t = wlo8pool.tile([KP, W_SLICE * NHP], e3)
            w8t = w8pool.tile([KP, W_SLICE * NHP], e3)
            lo, hi = ws * W_SLICE * NHP, (ws + 1) * W_SLICE * NHP
            nc.scalar.dma_start(w16t[:], w16f[:, lo:hi])
            nc.scalar.dma_start(wlo8t[:], wlo8f[:, lo:hi])
            nc.scalar.dma_start(w8t[:], w8f[:, lo:hi])
            if ws == 12:
                load_consts()

        xg16 = xpool16.tile([KP, MAXG * TBL], f16)
        xg8 = xpool8.tile([KP, MAXG * TBL], e3)
        gsz = gch * TBL
        nc.sync.dma_start(xg16[:, :gsz], xt16[:, c0 * TBL:(c0 + gch) * TBL])
        nc.scalar.dma_start(xg8[:, :gsz], xt8[:, c0 * TBL:(c0 + gch) * TBL])

        def wsl(t_, c):
            o = (c % W_SLICE) * NHP
            return t_[:, o:o + NHP]

        # T1 batch (psa), then T2+T3 batch (psb): psum write region
        # switches once per group, not per chunk.
        for ci in range(gch):
            c = c0 + ci
            nc.tensor.matmul(
                psa[:], lhsT=wsl(w16t, c),
                rhs=xg16[:, ci * TBL:(ci + 1) * TBL],
                start=(c == 0), stop=(c == KCH - 1))
        for ci in range(gch):
            c = c0 + ci
            nc.tensor.matmul(
                psb[:], lhsT=wsl(wlo8t, c),
                rhs=xg16[:, ci * TBL:(ci + 1) * TBL],
                start=(c == 0), stop=False)
        for ci in range(gch):
            c = c0 + ci
            nc.tensor.matmul(
                psb[:], lhsT=wsl(w8t, c),
                rhs=xg8[:, ci * TBL:(ci + 1) * TBL],
                start=False, stop=(c == KCH - 1))
        c0 += gch

    # cur1 table: curt = (psb*COMB + b1) + psa   (one PSUM read per op)
    nc.vector.tensor_scalar(
        curt[0:NH, 0:TBL], psb[0:NH, :], COMB, b1sb[:, 0:1],
        Alu.mult, Alu.add)
    nc.vector.tensor_add(curt[0:NH, 0:TBL], curt[0:NH, 0:TBL], psa[0:NH, :])

    # threshold table: thtab[h, blk t] = -beta1 - cur_{t+1}
    nc.vector.tensor_scalar(
        thtab[0:NH, 0:NSTEP * BL], curt[0:NH, BL:(NSTEP + 1) * BL], -1.0,
        betnsb[:, 0:1], Alu.mult, Alu.add)
    # u_0 = [-cur_0 ; 0]
    nc.vector.tensor_scalar(
        utab[0:NH, 0:BL], curt[0:NH, 0:BL], -1.0, None, Alu.mult)

    # ---- unified scan ----
    pss = ps_s.tile([KP, NSTEP * BL], f32)
    # preload psum with the shifted cur table (d1 accumulates onto it)
    nc.tensor.matmul(pss[:], lhsT=id32sb[:], rhs=curt[:, BL:(NSTEP + 1) * BL],
                     start=True, stop=False, skip_group_check=True)
    for t in range(NSTEP):
        blk = slice(t * BL, (t + 1) * BL)
        nblk = slice((t + 1) * BL, (t + 2) * BL)
        nc.tensor.matmul(pss[:, blk], lhsT=a16sb[:], rhs=spk1buf[:, blk],
                         start=False, stop=False, skip_group_check=True)
        nc.tensor.matmul(pss[:, blk], lhsT=alosb[:], rhs=spk1buf[:, blk],
                         start=False, stop=True, skip_group_check=True)
        # u_{t+1} = beta*u_t - d1_t
        nc.vector.scalar_tensor_tensor(
            utab[0:NU, nblk], utab[0:NU, blk], bcolsb[:, 0:1], pss[0:NU, blk],
            Alu.mult, Alu.subtract)
        # s_{t+1} = (u_{t+1} < thtab_t)
        nc.vector.tensor_tensor(
            spk1buf[0:NU, nblk], utab[0:NU, nblk], thtab[0:NU, blk], Alu.is_lt)

    # mem2_t = -u2_{t+2} / BETA2 (32-aligned read from partition 96;
    # rows 0..3 of memout are garbage and not DMA'd)
    nc.vector.tensor_scalar(
        memout[:], utab[96:NU, 2 * BL:(T + 2) * BL], -1.0 / BETA2, None,
        Alu.mult)
    nc.sync.dma_start(spk_o[:], spk1buf[NH:NU, 2 * BL:(T + 2) * BL])
    nc.sync.dma_start(mem_o[:], memout[4:4 + NOUT, :])
    stack.close()


def build_program():
    if "prog" in _PROG:
        return _PROG["prog"]
    import concourse.tile as tile
    from concourse import bacc, mybir

    f32 = mybir.dt.float32
    f16 = mybir.dt.float16
    e3 = mybir.dt.float8e3
    nc = bacc.Bacc("TRN2", target_bir_lowering=False, debug=False,
                   num_devices=NCORES)
    xt16 = nc.dram_tensor("xt16", [KP, KCH * TBL], f16,
                          kind="ExternalInput").ap()
    xt8 = nc.dram_tensor("xt8", [KP, KCH * TBL], e3,
                         kind="ExternalInput").ap()
    w16f = nc.dram_tensor("w16f", [KP, KCH * NHP], f16,
                          kind="ExternalInput").ap()
    wlo8f = nc.dram_tensor("wlo8f", [KP, KCH * NHP], e3,
                           kind="ExternalInput").ap()
    w8f = nc.dram_tensor("w8f", [KP, KCH * NHP], e3,
                         kind="ExternalInput").ap()
    a16d = nc.dram_tensor("a16d", [KP, KP], f16, kind="ExternalInput").ap()
    alod = nc.dram_tensor("alod", [KP, KP], f16, kind="ExternalInput").ap()
    id32d = nc.dram_tensor("id32d", [KP, KP], f32, kind="ExternalInput").ap()
    bcold = nc.dram_tensor("bcold", [NU, 1], f32, kind="ExternalInput").ap()
    betnd = nc.dram_tensor("betnd", [NH, 1], f32, kind="ExternalInput").ap()
    b1d = nc.dram_tensor("b1d", [NH, 1], f32, kind="ExternalInput").ap()
    s1initd = nc.dram_tensor("s1initd", [KP, SBLK * BL], f16,
                             kind="ExternalInput").ap()
    spk_o = nc.dram_tensor("spk", [NOUT, TBL], f16,
                           kind="ExternalOutput").ap()
    mem_o = nc.dram_tensor("mem", [NOUT, TBL], f32,
                           kind="ExternalOutput").ap()
    aps = (xt16, xt8, w16f, wlo8f, w8f, a16d, alod, id32d, bcold, betnd,
           b1d, s1initd, spk_o, mem_o)
    with tile.TileContext(nc) as tc:
        _build_body(tc, nc, mybir, aps)
    nc.compile()
    _PROG["prog"] = nc
    return nc


def _chunk_major(kxn):
    """[K=NIN, N] -> [128, KCH, N] (chunk-major matmul layout)."""
    n = kxn.shape[1]
    return np.ascontiguousarray(
        kxn.reshape(KCH, KP, n).transpose(1, 0, 2))


def prep_inputs(x, W1, b1, beta1, V, Vb, W2, b2):
    """Host-side shard + layout prep. Returns list of per-core input dicts."""
    import ml_dtypes

    f32 = np.float32
    f16 = np.float16
    e3 = ml_dtypes.float8_e3m4

    w1tp = np.zeros((NIN, NHP), f32)                     # NH padded to 128
    w1tp[:, :NH] = W1.T
    w16 = w1tp.astype(f16)
    wlo8 = ((w1tp - w16.astype(f32)) * SC_WLO).astype(e3)
    w8 = (w1tp * SC_W8).astype(e3)
    w16f = _chunk_major(w16).reshape(KP, KCH * NHP)
    wlo8f = _chunk_major(wlo8).reshape(KP, KCH * NHP)
    w8f = _chunk_major(w8).reshape(KP, KCH * NHP)

    beta1c = np.clip(beta1, 0.0, 1.0).astype(f32)
    # unified scan matrix A[k, m]
    A = np.zeros((KP, KP), f32)
    A[0:NH, 0:NH] = ((V - THRESH * np.eye(NH, dtype=f32))
                     * beta1c[:, None]).T
    A[NH + NOUT + 1, 0:NH] = beta1c * Vb          # layer-1 bias row (111)
    A[0:NH, NH:NU] = np.float32(BETA2) * W2.T
    A[NH:NU, NH:NU] = -np.float32(BETA2) * np.eye(NOUT, dtype=f32)
    A[NH + NOUT + 2, NH:NU] = np.float32(BETA2) * b2   # layer-2 bias row (112)
    a16 = A.astype(f16)
    alo = (A - a16.astype(f32)).astype(f16)       # subnormals, HW-exact
    id32 = np.eye(KP, dtype=f32)
    bcol = np.concatenate([beta1c, np.full(NOUT, BETA2, f32)]).reshape(NU, 1)
    betn = (-beta1c).reshape(NH, 1)
    b1a = np.ascontiguousarray(b1.reshape(NH, 1), dtype=f32)
    s1init = np.zeros((KP, SBLK * BL), f16)
    s1init[NH + NOUT + 1] = 1.0                   # layer-1 bias row
    s1init[NH + NOUT + 2, BL:] = 1.0              # layer-2 bias, off at t=0

    # x: [T, B, NIN] -> per-core matmul-ready chunk-major fp16 + e3m4 planes
    xt_full = np.ascontiguousarray(x.transpose(2, 0, 1))        # [NIN, T, B]
    in_maps = []
    for c in range(NCORES):
        xTc = np.ascontiguousarray(
            xt_full[:, :, c * BL:(c + 1) * BL]).reshape(NIN, TBL)
        v = _chunk_major(xTc)                            # [128, KCH, TBL]
        v16 = v.astype(f16)
        vlo8 = ((v - v16.astype(f32)) * SC_XLO).astype(e3)
        m = dict(w16f=w16f, wlo8f=wlo8f, w8f=w8f, a16d=a16, alod=alo,
                 id32d=id32, bcold=bcol, betnd=betn, b1d=b1a,
                 s1initd=s1init,
                 xt16=v16.reshape(KP, KCH * TBL),
                 xt8=vlo8.reshape(KP, KCH * TBL))
        in_maps.append(m)
    return in_maps


def gather_outputs(results):
    """results: list of per-core {'spk': [NOUT, TBL] f16, 'mem': [NOUT, TBL]}."""
    spks, mems = [], []
    for r in results:
        spks.append(np.ascontiguousarray(
            r["spk"].astype(np.float32).reshape(NOUT, T, BL).transpose(1, 2, 0)))
        mems.append(np.ascontiguousarray(
            r["mem"].reshape(NOUT, T, BL).transpose(1, 2, 0)))
    spk = np.concatenate(spks, axis=1)
    mem = np.concatenate(mems, axis=1)
    return spk.astype(np.float32), mem.astype(np.float32)


def kernel(x, W1, b1, beta1, V, Vb, W2, b2, **_run_kwargs):
    from concourse import bass_utils

    nc = build_program()
    in_maps = prep_inputs(np.asarray(x, np.float32), np.asarray(W1, np.float32),
                          np.asarray(b1, np.float32), np.asarray(beta1, np.float32),
                          np.asarray(V, np.float32), np.asarray(Vb, np.float32),
                          np.asarray(W2, np.float32), np.asarray(b2, np.float32))
    res = bass_utils.run_bass_kernel_spmd(
        nc, in_maps, core_ids=list(range(NCORES)), **_run_kwargs)
    out = gather_outputs(res.results)
    kernel.last_result = res
    return out


# revision 16
# speedup vs baseline: 1.1403x; 1.1403x over previous
"""Trainium2 Bass kernel for nn_Net_34359738709 (spiking RNN).

Model (per timestep t, reference semantics):
    cur1  = x_t @ W1.T + b1                      # [B, NH] big matmul, t-independent
    mem1  = beta1c*mem1 + cur1 + spk1 @ V.T + Vb - spk1*THRESH
    spk1  = (mem1 - THRESH > 0)
    cur2  = spk1 @ W2.T + b2
    mem2  = BETA2*mem2 + cur2 - spk2_prev*THRESH
    spk2  = (mem2 - THRESH > 0)
outputs: (spk2_rec, mem2_rec), each [T, B, NOUT]

== Stream phase ==
Data-parallel over batch (B=64 -> 8 cores x 8). cur1.T[NH, 400] is ONE
400-column tile accumulated over 256 K-chunks of 128 in PSUM. Precision
scheme (numpy-validated exact spike pattern, margin 1.2e-5):
    x   = fp16(x)  +  2^-12 * e3m4((x - fp16(x)) * 2^12)     3 B/elem
    W1  = fp16(W1) + 2^-21 * e3m4((W1 - fp16(W1)) * 2^21)    4 B/elem
    T1 = w16.T @ x16 (fp16);  T2 = wlo8.T @ x16 (e3m4 lhsT x fp16 rhs,
    mixed dtype);  T3 = w8.T @ xlo8 (e3m4, w8 = e3m4(W1*2^9)).
T2/T3 share scale 2^21 -> one shared psum; cur1 = psA + 2^-21*psB + b1.
W1 planes are NH-padded to 128 columns so LDWEIGHTS hits the
compiler's Fast-Weight-Load path, and streamed through 16-chunk pool
slices on the scalar ring; x16+xlo8 stream on the sync ring.

== Scan phase (unified, 51 steps) ==
Both layers run in ONE recurrence on a 128-row state vector
s = [spk1(0..99); spk2(100..110); 1(111); step>0(112)]:
    A[:, 0:100]   = beta1c * [(V-I).T; Vb at row 111]          (layer 1)
    A[:, 100:111] = BETA2 * [W2.T; -I at rows 100..110; b2 at row 112]
    (layer 2 rides at lag 1: its "input current" c2 - reset arrives
     through the SAME matmul; row 112 gates b2 off at t=0)
    u_{t+1} = beta (.) u_t - d1_t,   d1_t = A.T @ s_t + cur_{t+1}
    s_{t+1} = (u_{t+1} < -beta - cur_{t+1})     [one is_lt vs a table]
    mem2_t  = -u2_{t+2} / BETA2                  [recovered post-loop]
Per step: one fp16 matmul pair (A16 + Alo, Alo held as raw fp16
SUBNORMALS - HW-verified exact) accumulating onto a cur-preloaded PSUM
(preload = one fp32 identity matmul), then TWO DVE ops. No fp32 scan
matmuls, no separate layer-2 chain, no W2 bursts, no mem1 tensor.

Negative results (measured, don't re-try): 3-column-tile pipeline
(228us: 3x LDWEIGHTS re-tax, scan convoys block later tiles' MMs on
the in-order PE queue; any multi-tile split loses); fp32 scan matmul
(2-pass lowering, 288-338ns LDW + 353ns MM per step); layer-2 lag-8
interleave on DVE (6 DVE ops/step -> 1.3us/step); layer-1/layer-2
fixpoint-iteration scans (46/12 iterations on the seed data - causal
spike chains are deep); bf16x2 4B/elem split (233us); unpadded NH=100
stationary tiles; scan steps interleaved into MM groups; GpSimd.
"""

import sys

if "/opt/trn_rl_repo" not in sys.path:
    sys.path.insert(0, "/opt/trn_rl_repo")

import numpy as np

# Problem shapes (hardcoded per contract)
T, B, NIN, NH, NOUT = 50, 64, 32768, 100, 11
NCORES = 8
BL = B // NCORES          # 8 batch rows per core
TBL = T * BL              # 400 columns (t-major: col = t*BL + b)
KP = 128                  # contraction partition size
NHP = 128                 # NH padded to 128 (stationary tile width for FWL)
KCH = NIN // KP           # 256 K-chunks
X_GROUPS = [2, 2, 4] + [8] * 31   # K-chunks per x dma_start (sums to 256)
W_SLICE = 16              # K-chunks per W1 dma slice
THRESH = 1.0
BETA2 = 0.9753
NU = NH + NOUT            # 111 unified state rows
NSTEP = T + 1             # 51 scan steps (last one finishes layer 2)
SBLK = NSTEP + 1          # 52 column blocks in scan tables

SC_XLO = 2.0 ** 12        # xlo8 = e3m4((x - x16) * SC_XLO)
SC_WLO = 2.0 ** 21        # wlo8 = e3m4((W1 - w16) * SC_WLO)
SC_W8 = 2.0 ** 9          # w8   = e3m4(W1 * SC_W8); SC_W8*SC_XLO == SC_WLO
COMB = 1.0 / SC_WLO       # psumB combine scale

_PROG = {}


def _build_body(tc, nc, mybir, aps):
    f32 = mybir.dt.float32
    f16 = mybir.dt.float16
    e3 = mybir.dt.float8e3
    Alu = mybir.AluOpType
    (xt16, xt8, w16f, wlo8f, w8f, a16d, alod, id32d, bcold, betnd, b1d,
     s1initd, spk_o, mem_o) = aps

    from contextlib import ExitStack

    stack = ExitStack()
    const_pool = stack.enter_context(tc.tile_pool(name="const", bufs=1))
    state_pool = stack.enter_context(tc.tile_pool(name="state", bufs=1))
    w16pool = stack.enter_context(tc.tile_pool(name="w16p", bufs=4))
    wlo8pool = stack.enter_context(tc.tile_pool(name="wlo8p", bufs=4))
    w8pool = stack.enter_context(tc.tile_pool(name="w8p", bufs=4))
    xpool16 = stack.enter_context(tc.tile_pool(name="xg16", bufs=12))
    xpool8 = stack.enter_context(tc.tile_pool(name="xg8", bufs=12))
    ps_a = stack.enter_context(tc.tile_pool(name="psa", bufs=1, space="PSUM"))
    ps_b = stack.enter_context(tc.tile_pool(name="psb", bufs=1, space="PSUM"))
    ps_s = stack.enter_context(tc.tile_pool(name="pss", bufs=1, space="PSUM"))

    MAXG = max(X_GROUPS)

    # ---- scan constants / tables ----
    a16sb = const_pool.tile([KP, KP], f16)    # unified A matrix, fp16 hi
    alosb = const_pool.tile([KP, KP], f16)    # fp16 lo (subnormals)
    id32sb = const_pool.tile([KP, KP], f32)   # identity (psum preload)
    bcolsb = const_pool.tile([NU, 1], f32)    # [beta1c; BETA2] positive
    betnsb = const_pool.tile([NH, 1], f32)    # -beta1c
    b1sb = const_pool.tile([NH, 1], f32)
    spk1buf = state_pool.tile([KP, SBLK * BL], f16)  # unified spike state
    curt = state_pool.tile([KP, SBLK * BL], f32)     # cur1 table (+0 pad)
    thtab = state_pool.tile([NU, SBLK * BL], f32)    # threshold table
    utab = state_pool.tile([NU, SBLK * BL], f32)     # u state per step
    memout = state_pool.tile([4 + NOUT, TBL], f32)  # rows 4..14 are mem2

    def load_consts():
        nc.scalar.dma_start(a16sb[:], a16d)
        nc.scalar.dma_start(alosb[:], alod)
        nc.scalar.dma_start(id32sb[:], id32d)
        nc.scalar.dma_start(bcolsb[:], bcold)
        nc.scalar.dma_start(betnsb[:], betnd)
        nc.scalar.dma_start(b1sb[:], b1d)
        nc.scalar.dma_start(spk1buf[:], s1initd)

    # zero the pad regions of curt once (rows 100..127 everywhere, and
    # the two trailing column blocks of the cur rows)
    # (DVE partition starts must be 32-aligned; rows 96..99 of each
    # pad-memset are overwritten by the later 0:NH writes)
    nc.vector.memset(curt[96:KP, :], 0.0)
    nc.vector.memset(curt[0:NH, TBL:], 0.0)
    nc.vector.memset(thtab[96:NU, :], -BETA2)
    nc.vector.memset(utab[96:NU, 0:BL], 0.0)

    # ---- streaming matmul: one pass over 256 K-chunks, 400 columns ----
    psa = ps_a.tile([NHP, TBL], f32)
    psb = ps_b.tile([NHP, TBL], f32)
    w16t = wlo8t = w8t = None
    c0 = 0
    for g, gch in enumerate(X_GROUPS):
        if c0 % W_SLICE == 0:
            ws = c0 // W_SLICE
            w16t = w16pool.tile([KP, W_SLICE * NHP], f16)
            wlo8t = wlo8pool.tile([KP, W_SLICE * NHP], e3)
            w8t = w8pool.tile([KP, W_SLICE * NHP], e3)
            lo, hi = ws * W_SLICE * NHP, (ws + 1) * W_SLICE * NHP
            nc.scalar.dma_start(w16t[:], w16f[:, lo:hi])
            nc.scalar.dma_start(wlo8t[:], wlo8f[:, lo:hi])
            nc.scalar.dma_start(w8t[:], w8f[:, lo:hi])
            if ws == 12:
                load_consts()

        xg16 = xpool16.tile([KP, MAXG * TBL], f16)
        xg8 = xpool8.tile([KP, MAXG * TBL], e3)
        gsz = gch * TBL
        nc.sync.dma_start(xg16[:, :gsz], xt16[:, c0 * TBL:(c0 + gch) * TBL])
        nc.scalar.dma_start(xg8[:, :gsz], xt8[:, c0 * TBL:(c0 + gch) * TBL])

        def wsl(t_, c):
            o = (c % W_SLICE) * NHP
            return t_[:, o:o + NHP]

        # T1 batch (psa), then T2+T3 batch (psb): psum write region
        # switches once per group, not per chunk.
        for ci in range(gch):
            c = c0 + ci
            nc.tensor.matmul(
                psa[:], lhsT=wsl(w16t, c),
                rhs=xg16[:, ci * TBL:(ci + 1) * TBL],
                start=(c == 0), stop=(c == KCH - 1))
        for ci in range(gch):
            c = c0 + ci
            nc.tensor.matmul(
                psb[:], lhsT=wsl(wlo8t, c),
                rhs=xg16[:, ci * TBL:(ci + 1) * TBL],
                start=(c == 0), stop=False)
        for ci in range(gch):
            c = c0 + ci
            nc.tensor.matmul(
                psb[:], lhsT=wsl(w8t, c),
                rhs=xg8[:, ci * TBL:(ci + 1) * TBL],
                start=False, stop=(c == KCH - 1))
        c0 += gch

    # cur1 table: curt = (psb*COMB + b1) + psa   (one PSUM read per op)
    nc.vector.tensor_scalar(
        curt[0:NH, 0:TBL], psb[0:NH, :], COMB, b1sb[:, 0:1],
        Alu.mult, Alu.add)
    nc.vector.tensor_add(curt[0:NH, 0:TBL], curt[0:NH, 0:TBL], psa[0:NH, :])

    # threshold table: thtab[h, blk t] = -beta1 - cur_{t+1}
    nc.vector.tensor_scalar(
        thtab[0:NH, 0:NSTEP * BL], curt[0:NH, BL:(NSTEP + 1) * BL], -1.0,
        betnsb[:, 0:1], Alu.mult, Alu.add)
    # u_0 = [-cur_0 ; 0]
    nc.vector.tensor_scalar(
        utab[0:NH, 0:BL], curt[0:NH, 0:BL], -1.0, None, Alu.mult)

    # ---- unified scan ----
    pss = ps_s.tile([KP, NSTEP * BL], f32)
    # preload psum with the shifted cur table (d1 accumulates onto it)
    nc.tensor.matmul(pss[:], lhsT=id32sb[:], rhs=curt[:, BL:(NSTEP + 1) * BL],
                     start=True, stop=False, skip_group_check=True)
    for t in range(NSTEP):
        blk = slice(t * BL, (t + 1) * BL)
        nblk = slice((t + 1) * BL, (t + 2) * BL)
        nc.tensor.matmul(pss[:, blk], lhsT=a16sb[:], rhs=spk1buf[:, blk],
                         start=False, stop=False, skip_group_check=True)
        nc.tensor.matmul(pss[:, blk], lhsT=alosb[:], rhs=spk1buf[:, blk],
                         start=False, stop=True, skip_group_check=True)
        # u_{t+1} = beta*u_t - d1_t
        nc.vector.scalar_tensor_tensor(
            utab[0:NU, nblk], utab[0:NU, blk], bcolsb[:, 0:1], pss[0:NU, blk],
            Alu.mult, Alu.subtract)
        # s_{t+1} = (u_{t+1} < thtab_t)
        nc.vector.tensor_tensor(
            spk1buf[0:NU, nblk], utab[0:NU, nblk], thtab[0:NU, blk], Alu.is_lt)

    # mem2_t = -u2_{t+2} / BETA2 (32-aligned read from partition 96;
    # rows 0..3 of memout are garbage and not DMA'd)
    nc.vector.tensor_scalar(
        memout[:], utab[96:NU, 2 * BL:(T + 2) * BL], -1.0 / BETA2, None,
        Alu.mult)
    nc.sync.dma_start(spk_o[:], spk1buf[NH:NU, 2 * BL:(T + 2) * BL])
    nc.sync.dma_start(mem_o[:], memout[4:4 + NOUT, :])
    stack.close()


def build_program():
    if "prog" in _PROG:
        return _PROG["prog"]
    import concourse.tile as tile
    from concourse import bacc, mybir

    f32 = mybir.dt.float32
    f16 = mybir.dt.float16
    e3 = mybir.dt.float8e3
    nc = bacc.Bacc("TRN2", target_bir_lowering=False, debug=False,
                   num_devices=NCORES)
    xt16 = nc.dram_tensor("xt16", [KP, KCH * TBL], f16,
                          kind="ExternalInput").ap()
    xt8 = nc.dram_tensor("xt8", [KP, KCH * TBL], e3,
                         kind="ExternalInput").ap()
    w16f = nc.dram_tensor("w16f", [KP, KCH * NHP], f16,
                          kind="ExternalInput").ap()
    wlo8f = nc.dram_tensor("wlo8f", [KP, KCH * NHP], e3,
                           kind="ExternalInput").ap()
    w8f = nc.dram_tensor("w8f", [KP, KCH * NHP], e3,
                         kind="ExternalInput").ap()
    a16d = nc.dram_tensor("a16d", [KP, KP], f16, kind="ExternalInput").ap()
    alod = nc.dram_tensor("alod", [KP, KP], f16, kind="ExternalInput").ap()
    id32d = nc.dram_tensor("id32d", [KP, KP], f32, kind="ExternalInput").ap()
    bcold = nc.dram_tensor("bcold", [NU, 1], f32, kind="ExternalInput").ap()
    betnd = nc.dram_tensor("betnd", [NH, 1], f32, kind="ExternalInput").ap()
    b1d = nc.dram_tensor("b1d", [NH, 1], f32, kind="ExternalInput").ap()
    s1initd = nc.dram_tensor("s1initd", [KP, SBLK * BL], f16,
                             kind="ExternalInput").ap()
    spk_o = nc.dram_tensor("spk", [NOUT, TBL], f16,
                           kind="ExternalOutput").ap()
    mem_o = nc.dram_tensor("mem", [NOUT, TBL], f32,
                           kind="ExternalOutput").ap()
    aps = (xt16, xt8, w16f, wlo8f, w8f, a16d, alod, id32d, bcold, betnd,
           b1d, s1initd, spk_o, mem_o)
    with tile.TileContext(nc) as tc:
        _build_body(tc, nc, mybir, aps)
    nc.compile()
    _PROG["prog"] = nc
    return nc


def _chunk_major(kxn):
    """[K=NIN, N] -> [128, KCH, N] (chunk-major matmul layout)."""
    n = kxn.shape[1]
    return np.ascontiguousarray(
        kxn.reshape(KCH, KP, n).transpose(1, 0, 2))


def prep_inputs(x, W1, b1, beta1, V, Vb, W2, b2):
    """Host-side shard + layout prep. Returns list of per-core input dicts."""
    import ml_dtypes

    f32 = np.float32
    f16 = np.float16
    e3 = ml_dtypes.float8_e3m4

    w1tp = np.zeros((NIN, NHP), f32)                     # NH padded to 128
    w1tp[:, :NH] = W1.T
    w16 = w1tp.astype(f16)
    wlo8 = ((w1tp - w16.astype(f32)) * SC_WLO).astype(e3)
    w8 = (w1tp * SC_W8).astype(e3)
    w16f = _chunk_major(w16).reshape(KP, KCH * NHP)
    wlo8f = _chunk_major(wlo8).reshape(KP, KCH * NHP)
    w8f = _chunk_major(w8).reshape(KP, KCH * NHP)

    beta1c = np.clip(beta1, 0.0, 1.0).astype(f32)
    # unified scan matrix A[k, m]
    A = np.zeros((KP, KP), f32)
    A[0:NH, 0:NH] = ((V - THRESH * np.eye(NH, dtype=f32))
                     * beta1c[:, None]).T
    A[NH + NOUT + 1, 0:NH] = beta1c * Vb          # layer-1 bias row (111)
    A[0:NH, NH:NU] = np.float32(BETA2) * W2.T
    A[NH:NU, NH:NU] = -np.float32(BETA2) * np.eye(NOUT, dtype=f32)
    A[NH + NOUT + 2, NH:NU] = np.float32(BETA2) * b2   # layer-2 bias row (112)
    a16 = A.astype(f16)
    alo = (A - a16.astype(f32)).astype(f16)       # subnormals, HW-exact
    id32 = np.eye(KP, dtype=f32)
    bcol = np.concatenate([beta1c, np.full(NOUT, BETA2, f32)]).reshape(NU, 1)
    betn = (-beta1c).reshape(NH, 1)
    b1a = np.ascontiguousarray(b1.reshape(NH, 1), dtype=f32)
    s1init = np.zeros((KP, SBLK * BL), f16)
    s1init[NH + NOUT + 1] = 1.0                   # layer-1 bias row
    s1init[NH + NOUT + 2, BL:] = 1.0              # layer-2 bias, off at t=0

    # x: [T, B, NIN] -> per-core matmul-ready chunk-major fp16 + e3m4 planes
    xt_full = np.ascontiguousarray(x.transpose(2, 0, 1))        # [NIN, T, B]
    in_maps = []
    for c in range(NCORES):
        xTc = np.ascontiguousarray(
            xt_full[:, :, c * BL:(c + 1) * BL]).reshape(NIN, TBL)
        v = _chunk_major(xTc)                            # [128, KCH, TBL]
        v16 = v.astype(f16)
        vlo8 = ((v - v16.astype(f32)) * SC_XLO).astype(e3)
        m = dict(w16f=w16f, wlo8f=wlo8f, w8f=w8f, a16d=a16, alod=alo,
                 id32d=id32, bcold=bcol, betnd=betn, b1d=b1a,
                 s1initd=s1init,
                 xt16=v16.reshape(KP, KCH * TBL),
                 xt8=vlo8.reshape(KP, KCH * TBL))
        in_maps.append(m)
    return in_maps


def gather_outputs(results):
    """results: list of per-core {'spk': [NOUT, TBL] f16, 'mem': [NOUT, TBL]}."""
    spks, mems = [], []
    for r in results:
        spks.append(np.ascontiguousarray(
            r["spk"].astype(np.float32).reshape(NOUT, T, BL).transpose(1, 2, 0)))
        mems.append(np.ascontiguousarray(
            r["mem"].reshape(NOUT, T, BL).transpose(1, 2, 0)))
    spk = np.concatenate(spks, axis=1)
    mem = np.concatenate(mems, axis=1)
    return spk.astype(np.float32), mem.astype(np.float32)


def kernel(x, W1, b1, beta1, V, Vb, W2, b2, **_run_kwargs):
    from concourse import bass_utils

    nc = build_program()
    in_maps = prep_inputs(np.asarray(x, np.float32), np.asarray(W1, np.float32),
                          np.asarray(b1, np.float32), np.asarray(beta1, np.float32),
                          np.asarray(V, np.float32), np.asarray(Vb, np.float32),
                          np.asarray(W2, np.float32), np.asarray(b2, np.float32))
    res = bass_utils.run_bass_kernel_spmd(
        nc, in_maps, core_ids=list(range(NCORES)), **_run_kwargs)
    out = gather_outputs(res.results)
    kernel.last_result = res
    return out
